# revision 16
# baseline (speedup 1.0000x reference)
"""Trainium2 Bass kernel for nn_DenseGATGenerator.

Sharding: data-parallel over batch B=16 across 8 NeuronCores (2 elems/core).
All GEMM operands bf16 (fp32 PSUM accumulate); residual stream fp32.

Key design points (per batch element, token-major fp32 residual stream):
  - weights consumed in natural (K, M)/(K, N) layout as bf16; LN outputs are
    transposed once per phase on the PE so qkv/f1 produce feature-major
    intermediates and proj/f2 consume them as stationary operands.
  - pre-norm LN gains/biases folded into the following GEMM's weights/bias
    on the host; on-device LN is (x - mean) * rstd with a magic-seed Newton
    rsqrt on the VectorE.
  - LN statistics (bn_stats/bn_aggr) are emitted eagerly right after each
    residual tile update, so the next phase's LN has its stats ready and
    the PE pipeline doesn't stall at phase boundaries.
  - V is computed token-major straight from the qkv GEMM (stationary = xT
    block, moving = all of Wv) — no per-head PE transposes for V.
  - the additive per-head edge bias coef*A of encoder attention is
    accumulated into the score PSUM by an extra matmul with a
    coef-scaled identity as the stationary operand (A symmetric, bf16),
    so softmax reads PSUM directly with no DVE fixup pass.
  - attention computes TRANSPOSED scores sT = k q^T and exponentiates
    without max-subtraction; O contracts p @ [1 1 1 1 | v] on the PE so
    row-sums come from the same matmul; normalization = ScalarE Copy
    with a per-partition reciprocal scale (one batched reciprocal/head).
  - all 8 heads' score matmuls are emitted before any O matmul so the
    ScalarE exp of head h overlaps the PE scores of head h+1.
  - v-bias is folded into the proj bias on host (softmax rows sum to 1).
  - decoder exploits output symmetry: only column blocks >= row block are
    computed/stored; host reads the upper triangle only.
  - softplus = ln(1 + exp(x)); upper-triangle extraction on host.
"""

import numpy as np
from contextlib import ExitStack, contextmanager

import concourse.bass as bass
import concourse.mybir as mybir
import concourse.tile as tile
from concourse import bacc
from concourse.bass_utils import run_bass_kernel_spmd
from concourse.masks import make_identity

P = 128
D = 512
DT = D // P            # 4
NLR = 256
TE = NLR // P          # 2
NHR = 512
TH = NHR // P          # 4
NH = 8
HD = 64
FF = 2048
FFT = FF // P          # 16
L = 4
KDEC = 4
BE = 2                 # batch elems per core
NCORES = 8
B = 16
EPS = 1e-5
MAGIC = 0x5F3759DF

FP32 = mybir.dt.float32
F32R = mybir.dt.float32r
BF16 = mybir.dt.bfloat16
I32 = mybir.dt.int32
AF = mybir.ActivationFunctionType
ALU = mybir.AluOpType
AX = mybir.AxisListType


def _bcast(ap, parts=P):
    """Partition-broadcast a DRAM AP to [parts, ...] via stride-0."""
    return bass.AP(tensor=ap.tensor, offset=ap.offset, ap=[[0, parts], *ap.ap])


def build_nc():
    nc = bacc.Bacc()

    x_in = nc.declare_dram_parameter("X", [BE, NLR, NLR], BF16, isOutput=False)
    ab_in = nc.declare_dram_parameter("AB", [BE, NLR, NLR], BF16,
                                      isOutput=False)
    ipW = nc.declare_dram_parameter("ipW", [NLR, D], BF16, isOutput=False)
    qkvW = nc.declare_dram_parameter("qkvW", [L, D, 3 * D], BF16,
                                     isOutput=False)
    projW = nc.declare_dram_parameter("projW", [L, D, D], BF16,
                                      isOutput=False)
    f1W = nc.declare_dram_parameter("f1W", [L, D, FF], BF16, isOutput=False)
    f2W = nc.declare_dram_parameter("f2W", [L, FF, D], BF16, isOutput=False)
    up1W = nc.declare_dram_parameter("up1W", [NLR, NHR], BF16, isOutput=False)
    up2W = nc.declare_dram_parameter("up2W", [NHR, NHR], BF16, isOutput=False)
    rqkvW = nc.declare_dram_parameter("rqkvW", [D, 3 * D], BF16,
                                      isOutput=False)
    rprojW = nc.declare_dram_parameter("rprojW", [D, D], BF16, isOutput=False)
    rf1W = nc.declare_dram_parameter("rf1W", [D, FF], BF16, isOutput=False)
    rf2W = nc.declare_dram_parameter("rf2W", [FF, D], BF16, isOutput=False)
    decW = nc.declare_dram_parameter("decW", [KDEC, D, D], BF16,
                                     isOutput=False)
    ebc = nc.declare_dram_parameter("ebc", [L, 2 * D], BF16, isOutput=False)
    epp = nc.declare_dram_parameter("epp", [L, P, 36], FP32, isOutput=False)
    gbc = nc.declare_dram_parameter("gbc", [9 * D], BF16, isOutput=False)
    gpp = nc.declare_dram_parameter("gpp", [P, 37], FP32, isOutput=False)
    out_d = nc.declare_dram_parameter("OUT", [BE, NHR, NHR], FP32,
                                      isOutput=True)

    with TileKernel(nc) as tk:
        tk.run(x_in, ab_in, ipW, qkvW, projW, f1W, f2W, up1W, up2W,
               rqkvW, rprojW, rf1W, rf2W, decW, ebc, epp, gbc, gpp, out_d)

    nc.finalize()
    return nc


@contextmanager
def pool_group(tc, specs):
    with ExitStack() as st:
        yield [st.enter_context(
            tc.tile_pool(name=n, bufs=b, space=sp)
        ) for n, b, sp in specs]


class TileKernel:
    def __init__(self, nc):
        self.nc = nc
        self.ctx = ExitStack()

    def __enter__(self):
        self.tc = self.ctx.enter_context(tile.TileContext(self.nc))
        return self

    def __exit__(self, *exc):
        return self.ctx.__exit__(*exc)

    def pool(self, name, bufs, space="SBUF"):
        return self.ctx.enter_context(
            self.tc.tile_pool(name=name, bufs=bufs, space=space))

    # ---- layernorm statistics -------------------------------------------
    def emit_stats(self, mvs, t, src):
        """bn_stats+bn_aggr for one residual tile into mvs[:, t, :]."""
        nc = self.nc
        stats = self.small.tile([P, 6], FP32, tag="ln_stats", name="stats")
        nc.vector.bn_stats(stats[:, :], src)
        nc.vector.bn_aggr(mvs[:, t, :], stats[:, :])

    def make_stats(self, srcs, t_count):
        """Fresh mvs tile [P, t_count, 2] for sources without eager stats."""
        mvs = self.small.tile([P, t_count, 2], FP32, tag="ln_mvs", name="mvs")
        for t in range(t_count):
            self.emit_stats(mvs, t, srcs(t))
        return mvs

    # ---- single-elem layernorm: rsqrt chain + mixed-engine apply ---------
    def emit_ln(self, mvs, src_fn, out_tile, t_count, g_ap=None, b_ap=None):
        nc = self.nc
        small = self.small
        tc_ = t_count
        veps = small.tile([P, tc_], FP32, tag="ln_veps", name="veps")
        nc.vector.tensor_scalar(veps[:, :], mvs[:, :, 1], EPS, None,
                                op0=ALU.add)
        yi = small.tile([P, tc_], I32, tag="ln_yi0", name="yi")
        nc.vector.tensor_scalar(yi[:, :], veps[:, :].bitcast(I32),
                                self.one_i[:, :], None,
                                op0=ALU.arith_shift_right)
        nc.vector.tensor_tensor(yi[:, :], self.magic_i[:, 0:tc_], yi[:, :],
                                op=ALU.subtract)
        yt = small.tile([P, tc_], FP32, tag="ln_yi", name="yt")
        nc.vector.tensor_copy(yt[:, :], yi[:, :].bitcast(FP32))
        a = small.tile([P, tc_], FP32, tag="ln_a", name="a")
        for _ in range(3):
            nc.vector.tensor_tensor(a[:, :], veps[:, :], yt[:, :],
                                    op=ALU.mult)
            nc.vector.tensor_tensor(a[:, :], a[:, :], yt[:, :], op=ALU.mult)
            nc.vector.tensor_scalar(a[:, :], a[:, :], -0.5, 1.5,
                                    op0=ALU.mult, op1=ALU.add)
            nc.vector.tensor_tensor(yt[:, :], yt[:, :], a[:, :], op=ALU.mult)
        if g_ap is None:
            mb = small.tile([P, tc_], FP32, tag="ln_mb", name="mb")
            nc.vector.tensor_scalar(mb[:, :], mvs[:, :, 0], -1.0, None,
                                    op0=ALU.mult)
            nc.vector.tensor_tensor(mb[:, :], mb[:, :], yt[:, :],
                                    op=ALU.mult)
            for t in range(tc_):
                if t % 2 == 0:
                    nc.vector.tensor_scalar(
                        out_tile[:, t, :], src_fn(t), mvs[:, t, 0:1],
                        yt[:, t:t + 1], op0=ALU.subtract, op1=ALU.mult)
                else:
                    nc.scalar.activation(
                        out_tile[:, t, :], src_fn(t), AF.Identity,
                        bias=mb[:, t:t + 1], scale=yt[:, t:t + 1])
        else:
            for t in range(tc_):
                t2 = self.mid.tile([P, D], FP32, tag="ln_t2", name="t2")
                nc.vector.tensor_scalar(
                    t2[:, :], src_fn(t), mvs[:, t, 0:1],
                    yt[:, t:t + 1], op0=ALU.subtract, op1=ALU.mult)
                nc.vector.tensor_tensor(t2[:, :], t2[:, :], g_ap,
                                        op=ALU.mult)
                nc.vector.tensor_tensor(out_tile[:, t, :], t2[:, :], b_ap,
                                        op=ALU.add)

    # ---- layernorm apply (batched Newton rsqrt) --------------------------
    def ln_apply(self, jobs, t_count, g_ap=None, b_ap=None):
        """jobs: list of (mvs_tile, src_fn, out_tile).
        out[:, t, :] = (x - mean) * rstd [* g + b]."""
        nc = self.nc
        small = self.small
        nbt = len(jobs) * t_count
        veps = small.tile([P, nbt], FP32, tag="ln_veps", name="veps")
        for j, (mvs, _, _) in enumerate(jobs):
            nc.vector.tensor_scalar(
                veps[:, j * t_count:(j + 1) * t_count],
                mvs[:, :, 1], EPS, None, op0=ALU.add)
        yi = small.tile([P, nbt], I32, tag="ln_yi0", name="yi")
        nc.vector.tensor_scalar(yi[:, :], veps[:, :].bitcast(I32),
                                self.one_i[:, :], None,
                                op0=ALU.arith_shift_right)
        nc.vector.tensor_tensor(yi[:, :], self.magic_i[:, 0:nbt], yi[:, :],
                                op=ALU.subtract)
        yt = small.tile([P, nbt], FP32, tag="ln_yi", name="yt")
        nc.vector.tensor_copy(yt[:, :], yi[:, :].bitcast(FP32))
        a = small.tile([P, nbt], FP32, tag="ln_a", name="a")
        for _ in range(3):
            nc.vector.tensor_tensor(a[:, :], veps[:, :], yt[:, :],
                                    op=ALU.mult)
            nc.vector.tensor_tensor(a[:, :], a[:, :], yt[:, :], op=ALU.mult)
            nc.vector.tensor_scalar(a[:, :], a[:, :], -0.5, 1.5,
                                    op0=ALU.mult, op1=ALU.add)
            nc.vector.tensor_tensor(yt[:, :], yt[:, :], a[:, :], op=ALU.mult)
        for j, (mvs, src, out_tile) in enumerate(jobs):
            for t in range(t_count):
                i = j * t_count + t
                if g_ap is None:
                    nc.vector.tensor_scalar(
                        out_tile[:, t, :], src(t), mvs[:, t, 0:1],
                        yt[:, i:i + 1],
                        op0=ALU.subtract, op1=ALU.mult)
                else:
                    t2 = self.mid.tile([P, D], FP32, tag="ln_t2", name="t2")
                    nc.vector.tensor_scalar(
                        t2[:, :], src(t), mvs[:, t, 0:1],
                        yt[:, i:i + 1],
                        op0=ALU.subtract, op1=ALU.mult)
                    nc.vector.tensor_tensor(t2[:, :], t2[:, :], g_ap,
                                            op=ALU.mult)
                    nc.vector.tensor_tensor(out_tile[:, t, :], t2[:, :], b_ap,
                                            op=ALU.add)

    def transpose_group(self, ps_pool, src_fn, t_count, f_count, out_tile,
                        ps_tag="tr", ps_bufs=2):
        nc = self.nc
        for f in range(f_count):
            ps = ps_pool.tile([P, t_count * P], BF16, tag=ps_tag,
                              name="ps_tr", bufs=ps_bufs)
            for t in range(t_count):
                nc.tensor.transpose(ps[:, t * P:(t + 1) * P], src_fn(t, f),
                                    self.ident[:, :])
            if f % 2 == 0:
                nc.scalar.copy(out_tile[:, f, :], ps[:, :])
            else:
                nc.vector.tensor_copy(out_tile[:, f, :], ps[:, :])

    def mm(self, ps_ap, lhs_fn, rhs_fn, k_count):
        nc = self.nc
        for k in range(k_count):
            nc.tensor.matmul(ps_ap, lhs_fn(k), rhs_fn(k),
                             start=(k == 0), stop=(k == k_count - 1))

    # ---- model ----------------------------------------------------------
    def run(self, x_in, ab_in, ipW, qkvW, projW, f1W, f2W, up1W, up2W,
            rqkvW, rprojW, rf1W, rf2W, decW, ebc, epp, gbc, gpp, out_d):
        nc = self.nc
        tc = self.tc

        const = self.pool("const", 1)
        persist = self.pool("persist", 1)
        self.small = self.pool("small", 4)
        self.mid = self.pool("mid", 2)

        self.ident32 = const.tile([P, P], FP32)
        make_identity(nc, self.ident32[:, :])
        self.ident = const.tile([P, P], BF16)
        nc.vector.tensor_copy(self.ident[:, :], self.ident32[:, :])
        self.one_i = const.tile([P, 1], I32)
        nc.vector.memset(self.one_i[:, :], 1)
        self.magic_i = const.tile([P, BE * TH], I32)
        nc.vector.memset(self.magic_i[:, :], MAGIC)

        gpp_sb = persist.tile([P, 37], FP32)
        nc.sync.dma_start(out=gpp_sb[:, :], in_=gpp[:, :])

        hr_res = self.pool("hr_res", 1)
        h_hr = [hr_res.tile([P, TH, D], FP32, tag=f"Hhr{b}", name=f"Hhr{b}")
                for b in range(BE)]

        hand = self.pool("handoff", 1)
        x_hr0 = [hand.tile([P, TH, D], BF16, tag=f"xhr{b}", name=f"xhr{b}")
                 for b in range(BE)]
        hf2s = [hand.tile([P, TH, D], BF16, tag=f"hf2{b}", name=f"hf2{b}")
                for b in range(BE)]
        gbc_dec = hand.tile([P, 2, D], BF16, tag="gbc_dec")
        hr_w = self.pool("hr_w", 1)
        rqkvW_sb = hr_w.tile([P, DT, 3 * D], BF16, tag="qkvW")
        rprojW_sb = hr_w.tile([P, DT, D], BF16, tag="projW")
        gbc_hr = hr_w.tile([P, 2, D], BF16, tag="gbc_hr")

        def load_hr():
            nc.sync.dma_start(
                out=gbc_dec[:, :, :],
                in_=_bcast(gbc[7 * D:9 * D].rearrange("(a b) -> a b", b=D)))
            nc.sync.dma_start(
                out=rqkvW_sb[:, :, :],
                in_=rqkvW[:, :].rearrange("(k p) n -> p k n", p=P))
            nc.sync.dma_start(
                out=rprojW_sb[:, :, :],
                in_=rprojW[:, :].rearrange("(k p) n -> p k n", p=P))
            nc.sync.dma_start(
                out=gbc_hr[:, :, :],
                in_=_bcast(gbc[5 * D:7 * D].rearrange("(a b) -> a b", b=D)))

        with pool_group(tc, [("enc_res", 1, "SBUF"),
                             ("enc_misc", 1, "SBUF")]) \
                as (enc_res, enc_misc):
            h_enc = [enc_res.tile([P, TE, D], FP32, tag=f"Henc{b}",
                                  name=f"Henc{b}") for b in range(BE)]
            a_t = [enc_res.tile([P, TE, NLR], BF16, tag=f"A{b}", name=f"A{b}")
                   for b in range(BE)]
            for b in range(BE):
                nc.scalar.dma_start(
                    out=a_t[b][:, :, :],
                    in_=ab_in[b].rearrange("(t p) m -> p t m", p=P))

            up_ctx = ExitStack()
            up_w, up_sb = up_ctx.enter_context(pool_group(
                tc, [("up_w", 1, "SBUF"), ("up_sb", 1, "SBUF")]))
            gbc_en = up_w.tile([P, 2, D], BF16)
            up1W_sb = up_w.tile([P, TE, NHR], BF16)
            up2W_sb = up_w.tile([P, TH, NHR], BF16)
            hfs = [up_w.tile([P, TE, D], BF16, tag=f"hf{b}",
                             name=f"hf{b}") for b in range(BE)]

            enc_w_ctx = ExitStack()
            enc_w, enc_pk = enc_w_ctx.enter_context(pool_group(
                tc, [("enc_w", 1, "SBUF"), ("enc_pk", 1, "SBUF")]))

            def load_enc(l):
                w = {}
                w["qkv"] = enc_w.tile([P, DT, 3 * D], BF16, tag="qkvW",
                                      name="qkvW_sb", bufs=2)
                nc.sync.dma_start(
                    out=w["qkv"][:, :, :],
                    in_=qkvW[l].rearrange("(k p) n -> p k n", p=P))
                w["proj"] = enc_w.tile([P, DT, D], BF16, tag="projW",
                                       name="projW_sb", bufs=1)
                nc.sync.dma_start(
                    out=w["proj"][:, :, :],
                    in_=projW[l].rearrange("(k p) n -> p k n", p=P))
                w["f1"] = enc_w.tile([P, DT, FF], BF16, tag="f1W",
                                     name="f1W_sb", bufs=1)
                nc.sync.dma_start(
                    out=w["f1"][:, :, :],
                    in_=f1W[l].rearrange("(k p) n -> p k n", p=P))
                w["f2"] = enc_w.tile([P, FFT, D], BF16, tag="f2W",
                                     name="f2W_sb", bufs=1)
                nc.sync.dma_start(
                    out=w["f2"][:, :, :],
                    in_=f2W[l].rearrange("(k p) n -> p k n", p=P))
                w["ebc"] = enc_pk.tile([P, 2, D], BF16, tag="ebc",
                                       name="ebc_sb", bufs=1)
                nc.sync.dma_start(
                    out=w["ebc"][:, :, :],
                    in_=_bcast(ebc[l].rearrange("(a b) -> a b", b=D)))
                w["epp"] = enc_pk.tile([P, 36], FP32, tag="epp",
                                       name="epp_sb", bufs=2)
                nc.sync.dma_start(out=w["epp"][:, :], in_=epp[l])
                # coef-scaled identities for the edge-bias matmul
                w["cI"] = enc_pk.tile([P, NH, P], BF16, tag="cI",
                                      name="cI_sb", bufs=2)
                for h in range(NH):
                    nc.vector.tensor_scalar(
                        w["cI"][:, h, :], self.ident32[:, :],
                        w["epp"][:, 28 + h:29 + h], None, op0=ALU.mult)
                return w

            cur = load_enc(0)
            load_hr()
            nc.sync.dma_start(
                out=gbc_en[:, :, :],
                in_=_bcast(gbc[3 * D:5 * D].rearrange("(a b) -> a b", b=D)))
            nc.sync.dma_start(
                out=up1W_sb[:, :, :],
                in_=up1W[:, :].rearrange("(k p) n -> p k n", p=P))
            nc.sync.dma_start(
                out=up2W_sb[:, :, :],
                in_=up2W[:, :].rearrange("(k p) n -> p k n", p=P))

            with pool_group(tc, [("enc_a1", 1, "SBUF"),
                                 ("enc_a2", 2, "SBUF")]) as (act1, act2):
                xa = [act2.tile([P, TE, D], BF16, tag="ln_out",
                                name=f"xa_{b}", bufs=3) for b in range(BE)]

                # ---------------- phase 0: input projection ----------------
                with pool_group(tc, [("ip_sb", 1, "SBUF"),
                                     ("ip_ps", 2, "PSUM"),
                                     ("ip_w", 1, "SBUF")]) \
                        as (ip_sb, ip_ps, ip_w):
                    gbc_ip = ip_w.tile([P, 3, D], BF16)
                    nc.scalar.dma_start(
                        out=gbc_ip[:, :, :],
                        in_=_bcast(gbc[0:3 * D].rearrange("(a b) -> a b",
                                                          b=D)))
                    ipW_sb = ip_w.tile([P, TE, D], BF16)
                    nc.scalar.dma_start(
                        out=ipW_sb[:, :, :],
                        in_=ipW[:, :].rearrange("(k p) n -> p k n", p=P))
                    x_sbs = []
                    for b in range(BE):
                        x_sb = ip_sb.tile([P, TE, NLR], BF16, tag=f"x{b}",
                                          name=f"x{b}")
                        nc.scalar.dma_start(
                            out=x_sb[:, :, :],
                            in_=x_in[b].rearrange("(t p) m -> p t m", p=P))
                        x_sbs.append(x_sb)
                    zs = []
                    for b in range(BE):
                        xt = ip_sb.tile([P, TE, NLR], BF16, tag="xt",
                                        name="xt")
                        self.transpose_group(
                            ip_ps,
                            lambda t, f, b=b:
                                x_sbs[b][:, t, f * P:(f + 1) * P],
                            TE, TE, xt)
                        z = ip_sb.tile([P, TE, D], BF16, tag=f"z{b}",
                                       name=f"z{b}")
                        for m in range(TE):
                            ps = ip_ps.tile([P, D], FP32, tag="mm",
                                            name="ps")
                            self.mm(ps[:, :],
                                    lambda k: xt[:, k, m * P:(m + 1) * P],
                                    lambda k: ipW_sb[:, k, :], TE)
                            nc.vector.tensor_tensor(z[:, m, :], ps[:, :],
                                                    gbc_ip[:, 0, :],
                                                    op=ALU.add)
                        zs.append(z)
                    for b in range(BE):
                        lns = ip_sb.tile([P, TE, D], BF16, tag=f"lnout{b}",
                                         name=f"lnout{b}")
                        mvs_z = self.make_stats(
                            (lambda t, z=zs[b]: z[:, t, :]), TE)
                        self.emit_ln(mvs_z,
                                     (lambda t, z=zs[b]: z[:, t, :]),
                                     lns, TE,
                                     gbc_ip[:, 1, :], gbc_ip[:, 2, :])
                        mvs_h = self.small.tile([P, TE, 2], FP32,
                                                tag=f"mvh{b}",
                                                name=f"mvh{b}", bufs=4)
                        for t in range(TE):
                            nc.scalar.activation(h_enc[b][:, t, :],
                                                 lns[:, t, :], AF.Gelu)
                            self.emit_stats(mvs_h, t, h_enc[b][:, t, :])
                        self.emit_ln(mvs_h,
                                     (lambda t, b=b: h_enc[b][:, t, :]),
                                     xa[b], TE)

                # ---------------- encoder layers ----------------
                ly_ctx = ExitStack()
                ly_pools = ly_ctx.enter_context(pool_group(
                    tc, [("ly_mm", 2, "PSUM"), ("ly_s", 2, "PSUM"),
                         ("ly_o", 2, "PSUM"), ("ly_tr", 2, "PSUM")]))
                for l in range(L):
                    w = cur
                    if l + 1 < L:
                        cur = load_enc(l + 1)
                    xf = [act2.tile([P, TE, D], BF16, tag="ln_out",
                                    name=f"xf{l}_{b}", bufs=3)
                          for b in range(BE)]
                    self.attn_phase(
                        act1, act2, TE, h_enc, xa,
                        w["qkv"], w["proj"],
                        qkvb_cols=w["epp"][:, 0:12],
                        projb=w["ebc"][:, 0, :],
                        a_list=a_t, cI=w["cI"],
                        emit_next=lambda b, mvs, xf=xf: self.emit_ln(
                            mvs, (lambda t, b=b: h_enc[b][:, t, :]),
                            xf[b], TE),
                        pools=ly_pools)
                    if l + 1 < L:
                        xa = [act2.tile([P, TE, D], BF16, tag="ln_out",
                                        name=f"xa{l+1}_{b}", bufs=3)
                              for b in range(BE)]

                        def mk_next(xa=xa):
                            return lambda b, mvs: self.emit_ln(
                                mvs, (lambda t, b=b: h_enc[b][:, t, :]),
                                xa[b], TE)
                        nxt = mk_next()
                    else:
                        def nxt(b, mvs):
                            self.emit_ln(
                                mvs, (lambda t, b=b: h_enc[b][:, t, :]),
                                hfs[b], TE,
                                gbc_en[:, 0, :], gbc_en[:, 1, :])
                    self.ffn_phase(
                        act1, act2, TE, h_enc, xf, w["f1"], w["f2"],
                        f1b_cols=w["epp"][:, 12:28], f2b=w["ebc"][:, 1, :],
                        emit_next=nxt, pools=ly_pools)

            enc_w_ctx.close()

            # ---------------- upsample (hfs ready from enc tail) ----------
            if True:
                up_ps = ly_pools[0]
                for b in range(BE):
                    mvs_u = self.small.tile([P, TH, 2], FP32, tag=f"mvr{b}",
                                            name=f"mvr{b}", bufs=4)
                    g1 = up_sb.tile([P, TH, D], BF16, tag="g1", name="g1")
                    for mh in range(TH):
                        ps = up_ps.tile([P, D], FP32, tag="mm", name="ps")
                        self.mm(ps[:, :],
                                lambda k: up1W_sb[:, k, mh * P:(mh + 1) * P],
                                lambda k: hfs[b][:, k, :], TE)
                        nc.scalar.activation(g1[:, mh, :], ps[:, :], AF.Gelu,
                                             bias=gpp_sb[:, mh:mh + 1])
                    for mh in range(TH):
                        ps = up_ps.tile([P, D], FP32, tag="mm", name="ps")
                        self.mm(ps[:, :],
                                lambda k: up2W_sb[:, k, mh * P:(mh + 1) * P],
                                lambda k: g1[:, k, :], TH)
                        nc.vector.tensor_scalar(
                            h_hr[b][:, mh, :], ps[:, :],
                            gpp_sb[:, 4 + mh:5 + mh], None, op0=ALU.add)
                        self.emit_stats(mvs_u, mh, h_hr[b][:, mh, :])
                    self.emit_ln(mvs_u,
                                 (lambda t, b=b: h_hr[b][:, t, :]),
                                 x_hr0[b], TH)
            ly_ctx.close()
            up_ctx.close()

        # ---------------- HR refinement block ----------------
        with pool_group(tc, [("hr_w2", 1, "SBUF"), ("hr_a1", 1, "SBUF"),
                             ("hr_a2", 2, "SBUF")]) as (hr_w2, act1, act2):
            rf1W_sb = hr_w2.tile([P, DT, FF], BF16, tag="f1W")
            nc.sync.dma_start(
                out=rf1W_sb[:, :, :],
                in_=rf1W[:, :].rearrange("(k p) n -> p k n", p=P))
            rf2W_sb = hr_w2.tile([P, FFT, D], BF16, tag="f2W")
            nc.sync.dma_start(
                out=rf2W_sb[:, :, :],
                in_=rf2W[:, :].rearrange("(k p) n -> p k n", p=P))
            xr = [act2.tile([P, TH, D], BF16, tag="ln_out", name=f"xr_{b}",
                            bufs=3) for b in range(BE)]
            self.attn_phase(
                act1, act2, TH, h_hr, x_hr0, rqkvW_sb, rprojW_sb,
                qkvb_cols=gpp_sb[:, 8:20],
                projb=gbc_hr[:, 0, :],
                emit_next=lambda b, mvs: self.emit_ln(
                    mvs, (lambda t, b=b: h_hr[b][:, t, :]), xr[b], TH))
            self.ffn_phase(
                act1, act2, TH, h_hr, xr, rf1W_sb, rf2W_sb,
                f1b_cols=gpp_sb[:, 20:36], f2b=gbc_hr[:, 1, :],
                emit_next=lambda b, mvs: self.emit_ln(
                    mvs, (lambda t, b=b: h_hr[b][:, t, :]), hf2s[b], TH,
                    gbc_dec[:, 0, :], gbc_dec[:, 1, :]))

        # ---------------- decoder (hf2s ready from HR tail) ----------------
        with pool_group(tc, [("dec_sb", 1, "SBUF"), ("dec_sb2", 2, "SBUF"),
                             ("dec_ps", 2, "PSUM")]) as \
                (dec_sb, dec_sb2, dec_ps):
            decW_sb = dec_sb.tile([P, KDEC, DT, D], BF16, tag="decW")
            nc.sync.dma_start(
                out=decW_sb[:, :, :, :],
                in_=decW[:, :, :].rearrange("kd (k p) m -> p kd k m", p=P))
            for b in range(BE):
                hft = dec_sb.tile([P, DT, NHR], BF16, tag="hft", name="hft")
                self.transpose_group(
                    dec_ps,
                    lambda t, f: hf2s[b][:, t, f * P:(f + 1) * P],
                    TH, DT, hft)
                m1t = dec_sb.tile([P, KDEC, DT, NHR], BF16, tag="m1t",
                                  name="m1t")
                for kd in range(KDEC):
                    for mi in range(DT):
                        ps = dec_ps.tile([P, NHR], FP32, tag="mm", name="ps")
                        self.mm(
                            ps[:, :],
                            lambda k, kd=kd, mi=mi:
                                decW_sb[:, kd, k, mi * P:(mi + 1) * P],
                            lambda k: hft[:, k, :], DT)
                        if (kd * DT + mi) % 2 == 0:
                            nc.vector.tensor_copy(m1t[:, kd, mi, :], ps[:, :])
                        else:
                            nc.scalar.copy(m1t[:, kd, mi, :], ps[:, :])
                out_sb = dec_sb2.tile([P, TH, NHR], FP32, tag="out",
                                      name="out_sb")
                for md in range(TH):
                    ncols = NHR - md * P
                    ps = dec_ps.tile([P, NHR], FP32, tag="ak", name="ps_ak")
                    cnt = 0
                    for kd in range(KDEC):
                        for k in range(DT):
                            nc.tensor.matmul(
                                ps[:, 0:ncols],
                                m1t[:, kd, k, md * P:(md + 1) * P],
                                hft[:, k, md * P:],
                                start=(cnt == 0),
                                stop=(cnt == KDEC * DT - 1))
                            cnt += 1
                    # softplus(x/K + b) = ln(1 + exp(x/K + b))
                    sp_e = self.mid.tile([P, NHR], FP32, tag="sp_e",
                                         name="sp_e")
                    nc.scalar.activation(sp_e[:, 0:ncols], ps[:, 0:ncols],
                                         AF.Exp,
                                         bias=gpp_sb[:, 36:37],
                                         scale=1.0 / KDEC)
                    nc.scalar.activation(out_sb[:, md, 0:ncols],
                                         sp_e[:, 0:ncols],
                                         AF.Ln, bias=1.0)
                    nc.sync.dma_start(
                        out=out_d[b].rearrange("(t p) m -> p t m", p=P)
                            [:, md, md * P:],
                        in_=out_sb[:, md, 0:ncols])

    # ---- attention phase (both batch elems) -------------------------------
    def attn_phase(self, act1, act2, T, h_list, x1s, qkvW_sb, projW_sb,
                   qkvb_cols, projb, a_list=None, cI=None, emit_next=None,
                   pools=None):
        nc = self.nc
        tc = self.tc
        N = T * P
        if T == TE:
            ps_specs = [("at_ps", 2, "PSUM"), ("at_s", 2, "PSUM"),
                        ("at_o", 2, "PSUM"), ("at_tr", 2, "PSUM")]
        else:
            ps_specs = [("at_ps", 2, "PSUM"), ("at_s", 3, "PSUM"),
                        ("at_o", 2, "PSUM"), ("at_tr", 1, "PSUM")]
        tr_bufs = 2 if T == TE else 1
        ctx = ExitStack()
        if pools is None:
            aps, spool, opool, trpool = ctx.enter_context(
                pool_group(tc, ps_specs))
        else:
            aps, spool, opool, trpool = pools
        with ctx:
            x1t = []
            for b in range(BE):
                xt = act2.tile([P, DT, N], BF16, tag="ln_t", name="x1t")
                self.transpose_group(
                    trpool, lambda t, f: x1s[b][:, t, f * P:(f + 1) * P],
                    T, DT, xt, ps_bufs=tr_bufs)
                x1t.append(xt)
            # V token-major: psum [tokens, 8*64], evict into vext with ones
            vexts = []
            for b in range(BE):
                vext = act2.tile([P, T, NH, HD + 4], BF16, tag="vext",
                                 name=f"vext{b}", bufs=2)
                nc.vector.memset(vext[:, :, :, 0:4], 1.0)
                for t in range(T):
                    ps = aps.tile([P, D], FP32, tag="mm", name="ps_v")
                    self.mm(ps[:, :],
                            lambda k, t=t: x1t[b][:, k, t * P:(t + 1) * P],
                            lambda k: qkvW_sb[:, k, 2 * D:3 * D], DT)
                    ps_h = ps[:, :].rearrange("p (h c) -> p h c", c=HD)
                    if t % 2 == 0:
                        nc.scalar.copy(vext[:, t, :, 4:], ps_h)
                    else:
                        nc.vector.tensor_copy(vext[:, t, :, 4:], ps_h)
                vexts.append(vext)
            # Q/K feature-major per head-pair
            qks = []
            for b in range(BE):
                qk = act2.tile([P, DT, 2, N], BF16, tag="qk", name=f"qk{b}",
                               bufs=2)
                for hp in range(NH // 2):
                    for j, mi in enumerate((hp, 4 + hp)):
                        ps = aps.tile([P, N], FP32, tag="mm", name="ps_qk")
                        self.mm(
                            ps[:, :],
                            lambda k, mi=mi:
                                qkvW_sb[:, k, mi * P:(mi + 1) * P],
                            lambda k: x1t[b][:, k, :], DT)
                        if j == 0:  # q: (x + bias) * hd^-0.5
                            nc.vector.tensor_scalar(
                                qk[:, hp, j, :], ps[:, :],
                                qkvb_cols[:, mi:mi + 1], HD ** -0.5,
                                op0=ALU.add, op1=ALU.mult)
                        else:
                            nc.vector.tensor_scalar(
                                qk[:, hp, j, :], ps[:, :],
                                qkvb_cols[:, mi:mi + 1], None, op0=ALU.add)
                qks.append(qk)
            # scores + O, pipelined per head-pair: emit scores(hp), then
            # O(hp-1) — the ScalarE exp of pair hp overlaps PE O of hp-1.
            o_sbs = []
            for b in range(BE):
                o_sb = act1.tile([P, T, D], BF16, tag="o_sb", name=f"o_sb{b}",
                                 bufs=2)
                o_sbs.append(o_sb)

            def emit_scores(b, hp):
                pt = act1.tile([P, 2, T, N], BF16, tag="pT",
                               name=f"pt{b}_{hp}", bufs=3)
                for hh in range(2):
                    h_idx = 2 * hp + hh
                    base = hh * HD
                    qa = qks[b][base:base + HD, hp, 0, :]
                    ka = qks[b][base:base + HD, hp, 1, :]
                    if T == TE:
                        ps_s = spool.tile([P, T, N], FP32, tag="s",
                                          name="ps_s")
                        for kk in range(T):
                            if a_list is not None:
                                nc.tensor.matmul(
                                    ps_s[:, kk, :],
                                    cI[:, h_idx, :], a_list[b][:, kk, :],
                                    start=(kk == 0), stop=False)
                                nc.tensor.matmul(
                                    ps_s[:, kk, :],
                                    ka[:, kk * P:(kk + 1) * P], qa,
                                    start=False, stop=(kk == T - 1))
                            else:
                                nc.tensor.matmul(
                                    ps_s[:, kk, :],
                                    ka[:, kk * P:(kk + 1) * P], qa,
                                    start=(kk == 0), stop=(kk == T - 1))
                        nc.scalar.activation(pt[:, hh, :, :],
                                             ps_s[:, :, :], AF.Exp)
                    else:
                        for kk in range(T):
                            ps_s = spool.tile([P, N], FP32, tag="s",
                                              name="ps_s", bufs=3)
                            nc.tensor.matmul(
                                ps_s[:, :],
                                ka[:, kk * P:(kk + 1) * P], qa,
                                start=True, stop=True)
                            nc.scalar.activation(
                                pt[:, hh, kk, :],
                                ps_s[:, :], AF.Exp)
                return pt

            def emit_o(b, hp, pt):
                for hh in range(2):
                    h_idx = 2 * hp + hh
                    ps_o = opool.tile([P, T, HD + 4], FP32, tag="o",
                                      name="ps_o")
                    for m in range(T):
                        for kk in range(T):
                            nc.tensor.matmul(
                                ps_o[:, m, :],
                                pt[:, hh, kk, m * P:(m + 1) * P],
                                vexts[b][:, kk, h_idx, :],
                                start=(m == 0 and kk == 0),
                                stop=(m == T - 1 and kk == T - 1))
                    rinv = self.small.tile([P, T, 1], FP32, tag="rinv",
                                           name="rinv", bufs=4)
                    nc.vector.reciprocal(rinv[:, :, :], ps_o[:, :, 0:1])
                    for m in range(T):
                        if (h_idx + m) % 2 == 0:
                            nc.scalar.mul(
                                o_sbs[b][:, m, h_idx * HD:(h_idx + 1) * HD],
                                ps_o[:, m, 4:], rinv[:, m, 0:1])
                        else:
                            nc.vector.tensor_scalar(
                                o_sbs[b][:, m, h_idx * HD:(h_idx + 1) * HD],
                                ps_o[:, m, 4:], rinv[:, m, 0:1], None,
                                op0=ALU.mult)

            prev = None
            for b in range(BE):
                for hp in range(NH // 2):
                    pt = emit_scores(b, hp)
                    if prev is not None:
                        emit_o(prev[0], prev[1], prev[2])
                    prev = (b, hp, pt)
            emit_o(prev[0], prev[1], prev[2])
            # o -> feature-major oT, then proj + residual (+ eager LN stats)
            mvs_out = [self.small.tile([P, T, 2], FP32, tag=f"mva{b}",
                                       name=f"mva{b}", bufs=4)
                       for b in range(BE)]
            for b in range(BE):
                ot = act1.tile([P, DT, N], BF16, tag="oT", name="ot")
                self.transpose_group(
                    trpool, lambda t, f: o_sbs[b][:, t, f * P:(f + 1) * P],
                    T, DT, ot, ps_bufs=tr_bufs)
                for m in range(T):
                    ps = aps.tile([P, D], FP32, tag="mm", name="ps_proj")
                    self.mm(ps[:, :],
                            lambda k: ot[:, k, m * P:(m + 1) * P],
                            lambda k: projW_sb[:, k, :], DT)
                    nc.vector.tensor_tensor(h_list[b][:, m, :],
                                            h_list[b][:, m, :], ps[:, :],
                                            op=ALU.add)
                    nc.vector.tensor_tensor(h_list[b][:, m, :],
                                            h_list[b][:, m, :], projb,
                                            op=ALU.add)
                    self.emit_stats(mvs_out[b], m, h_list[b][:, m, :])
                if emit_next is not None:
                    emit_next(b, mvs_out[b])

    # ---- FFN phase (both batch elems) -------------------------------------
    def ffn_phase(self, act1, act2, T, h_list, x2s, f1W_sb, f2W_sb,
                  f1b_cols, f2b, emit_next=None, pools=None):
        nc = self.nc
        tc = self.tc
        N = T * P
        ctx = ExitStack()
        if pools is None:
            fps, facc, trpool = ctx.enter_context(pool_group(
                tc, [("ff_ps", 2, "PSUM"), ("ff_acc", 1, "PSUM"),
                     ("ff_tr", 2, "PSUM")]))
            facc_tag = lambda m: (facc, f"facc{m}")
        else:
            aps, spool, opool, trpool = pools
            fps = spool
            facc_tag = lambda m: (opool, "o")
        with ctx:
            x2t = []
            for b in range(BE):
                xt = act2.tile([P, DT, N], BF16, tag="ln_t", name="x2t")
                self.transpose_group(
                    trpool, lambda t, f: x2s[b][:, t, f * P:(f + 1) * P],
                    T, DT, xt)
                x2t.append(xt)
            mvs_out = [self.small.tile([P, T, 2], FP32, tag=f"mvf{b}",
                                       name=f"mvf{b}", bufs=4)
                       for b in range(BE)]
            for b in range(BE):
                ps_f2 = [facc_tag(m)[0].tile([P, D], FP32,
                                             tag=facc_tag(m)[1],
                                             name=f"facc{m}")
                         for m in range(T)]
                half = FFT // 4
                for wave in range(4):
                    gt = act1.tile([P, half, N], BF16, tag="gT", name="gt")
                    for j in range(half):
                        mf = wave * half + j
                        ps = fps.tile([P, N], FP32, tag="s", name="ps_f1")
                        self.mm(
                            ps[:, :],
                            lambda k, mf=mf:
                                f1W_sb[:, k, mf * P:(mf + 1) * P],
                            lambda k: x2t[b][:, k, :], DT)
                        nc.scalar.activation(gt[:, j, :], ps[:, :], AF.Gelu,
                                             bias=f1b_cols[:, mf:mf + 1])
                    for m in range(T):
                        for j in range(half):
                            mf = wave * half + j
                            nc.tensor.matmul(
                                ps_f2[m][:, :], gt[:, j, m * P:(m + 1) * P],
                                f2W_sb[:, mf, :],
                                start=(mf == 0), stop=(mf == FFT - 1))
                for m in range(T):
                    nc.vector.tensor_tensor(h_list[b][:, m, :],
                                            h_list[b][:, m, :],
                                            ps_f2[m][:, :], op=ALU.add)
                    nc.vector.tensor_tensor(h_list[b][:, m, :],
                                            h_list[b][:, m, :], f2b,
                                            op=ALU.add)
                    self.emit_stats(mvs_out[b], m, h_list[b][:, m, :])
                if emit_next is not None:
                    emit_next(b, mvs_out[b])


# --------------------------------------------------------------------------
# host-side driver
# --------------------------------------------------------------------------
_CACHE = {}
_TRIU = np.triu_indices(NHR, k=1)


def _np(x):
    return np.ascontiguousarray(np.asarray(x, dtype=np.float32))


def _bf(x):
    import ml_dtypes
    return np.ascontiguousarray(np.asarray(x).astype(ml_dtypes.bfloat16))


def kernel(**inputs):
    res = run_on_device(inputs)
    full = np.concatenate([res.results[c]["OUT"] for c in range(NCORES)],
                          axis=0)  # (16, 512, 512)
    return np.ascontiguousarray(full[:, _TRIU[0], _TRIU[1]]).astype(np.float32)


def _fold_ln(g, b, w, bias):
    """(xn*g + b) @ w + bias  ==  xn @ (diag(g) w) + (bias + b @ w)."""
    w64 = w.astype(np.float64)
    w2 = (g.astype(np.float64)[:, None] * w64).astype(np.float32)
    b2 = (bias.astype(np.float64) + b.astype(np.float64) @ w64).astype(
        np.float32)
    return w2, b2


def run_on_device(inputs, **run_kwargs):
    if "nc" not in _CACHE:
        _CACHE["nc"] = build_nc()
    nc = _CACHE["nc"]

    inp = {k: _np(v) for k, v in inputs.items()}

    qkvW_f = np.empty_like(inp["e_qkvW"])
    qkvb_f = np.empty_like(inp["e_qkvb"])
    f1W_f = np.empty_like(inp["e_f1W"])
    f1b_f = np.empty_like(inp["e_f1b"])
    for l in range(L):
        qkvW_f[l], qkvb_f[l] = _fold_ln(inp["e_n1g"][l], inp["e_n1b"][l],
                                        inp["e_qkvW"][l], inp["e_qkvb"][l])
        f1W_f[l], f1b_f[l] = _fold_ln(inp["e_n2g"][l], inp["e_n2b"][l],
                                      inp["e_f1W"][l], inp["e_f1b"][l])
    rqkvW_f, rqkvb_f = _fold_ln(inp["r_n1g"], inp["r_n1b"],
                                inp["r_qkvW"], inp["r_qkvb"])
    rf1W_f, rf1b_f = _fold_ln(inp["r_n2g"], inp["r_n2b"],
                              inp["r_f1W"], inp["r_f1b"])

    # fold the v bias into the proj bias: softmax rows sum to 1, so
    # o_full = o_norm + bv and (o+bv)@W + b = o@W + (b + bv@W).
    eprojb_f = np.empty_like(inp["e_projb"])
    for l in range(L):
        eprojb_f[l] = (inp["e_projb"][l].astype(np.float64)
                       + qkvb_f[l][2 * D:].astype(np.float64)
                       @ inp["e_projW"][l].astype(np.float64)).astype(
            np.float32)
    rprojb_f = (inp["r_projb"].astype(np.float64)
                + rqkvb_f[2 * D:].astype(np.float64)
                @ inp["r_projW"].astype(np.float64)).astype(np.float32)

    ebc = np.stack([
        np.concatenate([eprojb_f[l], inp["e_f2b"][l]])
        for l in range(L)
    ])
    epp = np.stack([
        np.concatenate([
            qkvb_f[l].reshape(12, P).T,
            f1b_f[l].reshape(FFT, P).T,
            np.broadcast_to(inp["e_ebs"][l] * inp["e_ebW"][l], (P, NH)),
        ], axis=1)
        for l in range(L)
    ])
    gbc = np.concatenate([
        inp["ip_b"], inp["ip_g"], inp["ip_bt"], inp["encn_g"], inp["encn_b"],
        rprojb_f, inp["r_f2b"], inp["hrn_g"], inp["hrn_b"],
    ])
    gpp = np.concatenate([
        inp["up1b"].reshape(TH, P).T,
        inp["up2b"].reshape(TH, P).T,
        rqkvb_f.reshape(12, P).T,
        rf1b_f.reshape(FFT, P).T,
        np.broadcast_to(inp["dec_b"][0], (P, 1)),
    ], axis=1)
    dec_sym = 0.5 * (inp["dec_W"] + inp["dec_W"].transpose(0, 2, 1))
    # the transposed-score path uses A^T == A; guarantee symmetry
    a_sym = 0.5 * (inp["A_lr"] + inp["A_lr"].transpose(0, 2, 1))

    shared = {
        "ipW": _bf(inp["ip_W"]), "qkvW": _bf(qkvW_f),
        "projW": _bf(inp["e_projW"]),
        "f1W": _bf(f1W_f), "f2W": _bf(inp["e_f2W"]), "up1W": _bf(inp["up1W"]),
        "up2W": _bf(inp["up2W"]), "rqkvW": _bf(rqkvW_f),
        "rprojW": _bf(inp["r_projW"]),
        "rf1W": _bf(rf1W_f), "rf2W": _bf(inp["r_f2W"]),
        "decW": _bf(dec_sym),
        "ebc": _bf(ebc), "epp": np.ascontiguousarray(epp),
        "gbc": _bf(gbc), "gpp": np.ascontiguousarray(gpp),
    }
    in_maps = []
    for c in range(NCORES):
        m = dict(shared)
        m["X"] = _bf(inp["X_lr"][c * BE:(c + 1) * BE])
        m["AB"] = _bf(a_sym[c * BE:(c + 1) * BE])
        in_maps.append(m)

    return run_bass_kernel_spmd(nc, in_maps, list(range(NCORES)), **run_kwargs)


if __name__ == "__main__":
    import time
    t0 = time.time()
    nc = build_nc()
    print(f"build+finalize: {time.time() - t0:.1f}s, insts={len(nc.inst_map)}")


# revision 19
# speedup vs baseline: 1.0509x; 1.0509x over previous
"""Trainium2 Bass kernel for nn_DenseGATGenerator.

Sharding: data-parallel over batch B=16 across 8 NeuronCores (2 elems/core).
All GEMM operands bf16 (fp32 PSUM accumulate); residual stream fp32.

Key design points (per batch element, token-major fp32 residual stream):
  - weights consumed in natural (K, M)/(K, N) layout as bf16; LN outputs are
    transposed once per phase on the PE so qkv/f1 produce feature-major
    intermediates and proj/f2 consume them as stationary operands.
  - pre-norm LN gains/biases folded into the following GEMM's weights/bias
    on the host; on-device LN is (x - mean) * rstd with a magic-seed Newton
    rsqrt on the VectorE.
  - LN statistics (bn_stats/bn_aggr) are emitted eagerly right after each
    residual tile update, so the next phase's LN has its stats ready and
    the PE pipeline doesn't stall at phase boundaries.
  - V is computed token-major straight from the qkv GEMM (stationary = xT
    block, moving = all of Wv) — no per-head PE transposes for V.
  - the additive per-head edge bias coef*A of encoder attention is
    accumulated into the score PSUM by an extra matmul with a
    coef-scaled identity as the stationary operand (A symmetric, bf16),
    so softmax reads PSUM directly with no DVE fixup pass.
  - attention computes TRANSPOSED scores sT = k q^T and exponentiates
    without max-subtraction; O contracts p @ [1 1 1 1 | v] on the PE so
    row-sums come from the same matmul; normalization = ScalarE Copy
    with a per-partition reciprocal scale (one batched reciprocal/head).
  - all 8 heads' score matmuls are emitted before any O matmul so the
    ScalarE exp of head h overlaps the PE scores of head h+1.
  - v-bias is folded into the proj bias on host (softmax rows sum to 1).
  - decoder exploits output symmetry: only column blocks >= row block are
    computed/stored; host reads the upper triangle only.
  - softplus = ln(1 + exp(x)); upper-triangle extraction on host.
"""

import numpy as np
from contextlib import ExitStack, contextmanager

import concourse.bass as bass
import concourse.mybir as mybir
import concourse.tile as tile
from concourse import bacc
from concourse.bass_utils import run_bass_kernel_spmd
from concourse.masks import make_identity

P = 128
D = 512
DT = D // P            # 4
NLR = 256
TE = NLR // P          # 2
NHR = 512
TH = NHR // P          # 4
NH = 8
HD = 64
FF = 2048
FFT = FF // P          # 16
L = 4
KDEC = 4
BE = 2                 # batch elems per core
NCORES = 8
B = 16
EPS = 1e-5
MAGIC = 0x5F3759DF

FP32 = mybir.dt.float32
F32R = mybir.dt.float32r
BF16 = mybir.dt.bfloat16
I32 = mybir.dt.int32
AF = mybir.ActivationFunctionType
ALU = mybir.AluOpType
AX = mybir.AxisListType


def _bcast(ap, parts=P):
    """Partition-broadcast a DRAM AP to [parts, ...] via stride-0."""
    return bass.AP(tensor=ap.tensor, offset=ap.offset, ap=[[0, parts], *ap.ap])


def build_nc():
    nc = bacc.Bacc()

    x_in = nc.declare_dram_parameter("X", [BE, NLR, NLR], BF16, isOutput=False)
    ab_in = nc.declare_dram_parameter("AB", [BE, NLR, NLR], BF16,
                                      isOutput=False)
    ipW = nc.declare_dram_parameter("ipW", [NLR, D], BF16, isOutput=False)
    qkvW = nc.declare_dram_parameter("qkvW", [L, D, 3 * D], BF16,
                                     isOutput=False)
    projW = nc.declare_dram_parameter("projW", [L, D, D], BF16,
                                      isOutput=False)
    f1W = nc.declare_dram_parameter("f1W", [L, D, FF], BF16, isOutput=False)
    f2W = nc.declare_dram_parameter("f2W", [L, FF, D], BF16, isOutput=False)
    up1W = nc.declare_dram_parameter("up1W", [NLR, NHR], BF16, isOutput=False)
    up2W = nc.declare_dram_parameter("up2W", [NHR, NHR], BF16, isOutput=False)
    rqkvW = nc.declare_dram_parameter("rqkvW", [D, 3 * D], BF16,
                                      isOutput=False)
    rprojW = nc.declare_dram_parameter("rprojW", [D, D], BF16, isOutput=False)
    rf1W = nc.declare_dram_parameter("rf1W", [D, FF], BF16, isOutput=False)
    rf2W = nc.declare_dram_parameter("rf2W", [FF, D], BF16, isOutput=False)
    decW = nc.declare_dram_parameter("decW", [KDEC, D, D], BF16,
                                     isOutput=False)
    ebc = nc.declare_dram_parameter("ebc", [L, 2 * D], BF16, isOutput=False)
    epp = nc.declare_dram_parameter("epp", [L, P, 36], FP32, isOutput=False)
    gbc = nc.declare_dram_parameter("gbc", [9 * D], BF16, isOutput=False)
    gpp = nc.declare_dram_parameter("gpp", [P, 37], FP32, isOutput=False)
    out_d = nc.declare_dram_parameter("OUT", [BE, NHR, NHR], FP32,
                                      isOutput=True)

    with TileKernel(nc) as tk:
        tk.run(x_in, ab_in, ipW, qkvW, projW, f1W, f2W, up1W, up2W,
               rqkvW, rprojW, rf1W, rf2W, decW, ebc, epp, gbc, gpp, out_d)

    nc.finalize()
    return nc


@contextmanager
def pool_group(tc, specs):
    with ExitStack() as st:
        yield [st.enter_context(
            tc.tile_pool(name=n, bufs=b, space=sp)
        ) for n, b, sp in specs]


class TileKernel:
    def __init__(self, nc):
        self.nc = nc
        self.ctx = ExitStack()

    def __enter__(self):
        self.tc = self.ctx.enter_context(tile.TileContext(self.nc))
        return self

    def __exit__(self, *exc):
        return self.ctx.__exit__(*exc)

    def pool(self, name, bufs, space="SBUF"):
        return self.ctx.enter_context(
            self.tc.tile_pool(name=name, bufs=bufs, space=space))

    # ---- layernorm statistics -------------------------------------------
    def emit_stats(self, mvs, t, src):
        """bn_stats+bn_aggr for one residual tile into mvs[:, t, :]."""
        nc = self.nc
        stats = self.small.tile([P, 6], FP32, tag="ln_stats", name="stats")
        nc.vector.bn_stats(stats[:, :], src)
        nc.vector.bn_aggr(mvs[:, t, :], stats[:, :])

    def make_stats(self, srcs, t_count):
        """Fresh mvs tile [P, t_count, 2] for sources without eager stats."""
        mvs = self.small.tile([P, t_count, 2], FP32, tag="ln_mvs", name="mvs")
        for t in range(t_count):
            self.emit_stats(mvs, t, srcs(t))
        return mvs

    # ---- single-elem layernorm: rsqrt chain + mixed-engine apply ---------
    def emit_ln(self, mvs, src_fn, out_tile, t_count, g_ap=None, b_ap=None):
        nc = self.nc
        small = self.small
        tc_ = t_count
        veps = small.tile([P, tc_], FP32, tag="ln_veps", name="veps")
        nc.vector.tensor_scalar(veps[:, :], mvs[:, :, 1], EPS, None,
                                op0=ALU.add)
        yi = small.tile([P, tc_], I32, tag="ln_yi0", name="yi")
        nc.vector.tensor_scalar(yi[:, :], veps[:, :].bitcast(I32),
                                self.one_i[:, :], None,
                                op0=ALU.arith_shift_right)
        nc.vector.tensor_tensor(yi[:, :], self.magic_i[:, 0:tc_], yi[:, :],
                                op=ALU.subtract)
        yt = small.tile([P, tc_], FP32, tag="ln_yi", name="yt")
        nc.vector.tensor_copy(yt[:, :], yi[:, :].bitcast(FP32))
        a = small.tile([P, tc_], FP32, tag="ln_a", name="a")
        for _ in range(3):
            nc.vector.tensor_tensor(a[:, :], veps[:, :], yt[:, :],
                                    op=ALU.mult)
            nc.vector.tensor_tensor(a[:, :], a[:, :], yt[:, :], op=ALU.mult)
            nc.vector.tensor_scalar(a[:, :], a[:, :], -0.5, 1.5,
                                    op0=ALU.mult, op1=ALU.add)
            nc.vector.tensor_tensor(yt[:, :], yt[:, :], a[:, :], op=ALU.mult)
        if g_ap is None:
            mb = small.tile([P, tc_], FP32, tag="ln_mb", name="mb")
            nc.vector.tensor_scalar(mb[:, :], mvs[:, :, 0], -1.0, None,
                                    op0=ALU.mult)
            nc.vector.tensor_tensor(mb[:, :], mb[:, :], yt[:, :],
                                    op=ALU.mult)
            for t in range(tc_):
                if t % 2 == 0:
                    nc.vector.tensor_scalar(
                        out_tile[:, t, :], src_fn(t), mvs[:, t, 0:1],
                        yt[:, t:t + 1], op0=ALU.subtract, op1=ALU.mult)
                else:
                    nc.scalar.activation(
                        out_tile[:, t, :], src_fn(t), AF.Identity,
                        bias=mb[:, t:t + 1], scale=yt[:, t:t + 1])
        else:
            for t in range(tc_):
                t2 = self.mid.tile([P, D], FP32, tag="ln_t2", name="t2")
                nc.vector.tensor_scalar(
                    t2[:, :], src_fn(t), mvs[:, t, 0:1],
                    yt[:, t:t + 1], op0=ALU.subtract, op1=ALU.mult)
                nc.vector.tensor_tensor(t2[:, :], t2[:, :], g_ap,
                                        op=ALU.mult)
                nc.vector.tensor_tensor(out_tile[:, t, :], t2[:, :], b_ap,
                                        op=ALU.add)

    # ---- layernorm apply (batched Newton rsqrt) --------------------------
    def ln_apply(self, jobs, t_count, g_ap=None, b_ap=None):
        """jobs: list of (mvs_tile, src_fn, out_tile).
        out[:, t, :] = (x - mean) * rstd [* g + b]."""
        nc = self.nc
        small = self.small
        nbt = len(jobs) * t_count
        veps = small.tile([P, nbt], FP32, tag="ln_veps", name="veps")
        for j, (mvs, _, _) in enumerate(jobs):
            nc.vector.tensor_scalar(
                veps[:, j * t_count:(j + 1) * t_count],
                mvs[:, :, 1], EPS, None, op0=ALU.add)
        yi = small.tile([P, nbt], I32, tag="ln_yi0", name="yi")
        nc.vector.tensor_scalar(yi[:, :], veps[:, :].bitcast(I32),
                                self.one_i[:, :], None,
                                op0=ALU.arith_shift_right)
        nc.vector.tensor_tensor(yi[:, :], self.magic_i[:, 0:nbt], yi[:, :],
                                op=ALU.subtract)
        yt = small.tile([P, nbt], FP32, tag="ln_yi", name="yt")
        nc.vector.tensor_copy(yt[:, :], yi[:, :].bitcast(FP32))
        a = small.tile([P, nbt], FP32, tag="ln_a", name="a")
        for _ in range(3):
            nc.vector.tensor_tensor(a[:, :], veps[:, :], yt[:, :],
                                    op=ALU.mult)
            nc.vector.tensor_tensor(a[:, :], a[:, :], yt[:, :], op=ALU.mult)
            nc.vector.tensor_scalar(a[:, :], a[:, :], -0.5, 1.5,
                                    op0=ALU.mult, op1=ALU.add)
            nc.vector.tensor_tensor(yt[:, :], yt[:, :], a[:, :], op=ALU.mult)
        for j, (mvs, src, out_tile) in enumerate(jobs):
            for t in range(t_count):
                i = j * t_count + t
                if g_ap is None:
                    nc.vector.tensor_scalar(
                        out_tile[:, t, :], src(t), mvs[:, t, 0:1],
                        yt[:, i:i + 1],
                        op0=ALU.subtract, op1=ALU.mult)
                else:
                    t2 = self.mid.tile([P, D], FP32, tag="ln_t2", name="t2")
                    nc.vector.tensor_scalar(
                        t2[:, :], src(t), mvs[:, t, 0:1],
                        yt[:, i:i + 1],
                        op0=ALU.subtract, op1=ALU.mult)
                    nc.vector.tensor_tensor(t2[:, :], t2[:, :], g_ap,
                                            op=ALU.mult)
                    nc.vector.tensor_tensor(out_tile[:, t, :], t2[:, :], b_ap,
                                            op=ALU.add)

    def transpose_group(self, ps_pool, src_fn, t_count, f_count, out_tile,
                        ps_tag="tr", ps_bufs=2):
        nc = self.nc
        for f in range(f_count):
            ps = ps_pool.tile([P, t_count * P], BF16, tag=ps_tag,
                              name="ps_tr", bufs=ps_bufs)
            for t in range(t_count):
                nc.tensor.transpose(ps[:, t * P:(t + 1) * P], src_fn(t, f),
                                    self.ident[:, :])
            if f % 2 == 0:
                nc.scalar.copy(out_tile[:, f, :], ps[:, :])
            else:
                nc.vector.tensor_copy(out_tile[:, f, :], ps[:, :])

    def mm(self, ps_ap, lhs_fn, rhs_fn, k_count):
        nc = self.nc
        for k in range(k_count):
            nc.tensor.matmul(ps_ap, lhs_fn(k), rhs_fn(k),
                             start=(k == 0), stop=(k == k_count - 1))

    # ---- model ----------------------------------------------------------
    def run(self, x_in, ab_in, ipW, qkvW, projW, f1W, f2W, up1W, up2W,
            rqkvW, rprojW, rf1W, rf2W, decW, ebc, epp, gbc, gpp, out_d):
        nc = self.nc
        tc = self.tc

        const = self.pool("const", 1)
        persist = self.pool("persist", 1)
        self.small = self.pool("small", 4)
        self.mid = self.pool("mid", 2)

        self.ident32 = const.tile([P, P], FP32)
        make_identity(nc, self.ident32[:, :])
        self.ident = const.tile([P, P], BF16)
        nc.vector.tensor_copy(self.ident[:, :], self.ident32[:, :])
        self.one_i = const.tile([P, 1], I32)
        nc.vector.memset(self.one_i[:, :], 1)
        self.magic_i = const.tile([P, BE * TH], I32)
        nc.vector.memset(self.magic_i[:, :], MAGIC)

        gpp_sb = persist.tile([P, 37], FP32)
        nc.sync.dma_start(out=gpp_sb[:, :], in_=gpp[:, :])

        hr_res = self.pool("hr_res", 1)
        h_hr = [hr_res.tile([P, TH, D], FP32, tag=f"Hhr{b}", name=f"Hhr{b}")
                for b in range(BE)]

        hand = self.pool("handoff", 1)
        x_hr0 = [hand.tile([P, TH, D], BF16, tag=f"xhr{b}", name=f"xhr{b}")
                 for b in range(BE)]
        hf2s = [hand.tile([P, TH, D], BF16, tag=f"hf2{b}", name=f"hf2{b}")
                for b in range(BE)]
        gbc_dec = hand.tile([P, 2, D], BF16, tag="gbc_dec")
        hr_w = self.pool("hr_w", 1)
        rqkvW_sb = hr_w.tile([P, DT, 3 * D], BF16, tag="qkvW")
        rprojW_sb = hr_w.tile([P, DT, D], BF16, tag="projW")
        gbc_hr = hr_w.tile([P, 2, D], BF16, tag="gbc_hr")

        def load_hr():
            nc.sync.dma_start(
                out=gbc_dec[:, :, :],
                in_=_bcast(gbc[7 * D:9 * D].rearrange("(a b) -> a b", b=D)))
            nc.sync.dma_start(
                out=rqkvW_sb[:, :, :],
                in_=rqkvW[:, :].rearrange("(k p) n -> p k n", p=P))
            nc.sync.dma_start(
                out=rprojW_sb[:, :, :],
                in_=rprojW[:, :].rearrange("(k p) n -> p k n", p=P))
            nc.sync.dma_start(
                out=gbc_hr[:, :, :],
                in_=_bcast(gbc[5 * D:7 * D].rearrange("(a b) -> a b", b=D)))

        with pool_group(tc, [("enc_res", 1, "SBUF"),
                             ("enc_misc", 1, "SBUF")]) \
                as (enc_res, enc_misc):
            h_enc = [enc_res.tile([P, TE, D], FP32, tag=f"Henc{b}",
                                  name=f"Henc{b}") for b in range(BE)]
            a_t = [enc_res.tile([P, TE, NLR], BF16, tag=f"A{b}", name=f"A{b}")
                   for b in range(BE)]
            for b in range(BE):
                nc.scalar.dma_start(
                    out=a_t[b][:, :, :],
                    in_=ab_in[b].rearrange("(t p) m -> p t m", p=P))

            up_ctx = ExitStack()
            up_w, up_sb = up_ctx.enter_context(pool_group(
                tc, [("up_w", 1, "SBUF"), ("up_sb", 1, "SBUF")]))
            gbc_en = up_w.tile([P, 2, D], BF16)
            up1W_sb = up_w.tile([P, TE, NHR], BF16)
            up2W_sb = up_w.tile([P, TH, NHR], BF16)
            hfs = [up_w.tile([P, TE, D], BF16, tag=f"hf{b}",
                             name=f"hf{b}") for b in range(BE)]

            enc_w_ctx = ExitStack()
            enc_w, enc_pk = enc_w_ctx.enter_context(pool_group(
                tc, [("enc_w", 1, "SBUF"), ("enc_pk", 1, "SBUF")]))

            def load_enc(l):
                w = {}
                w["qkv"] = enc_w.tile([P, DT, 3 * D], BF16, tag="qkvW",
                                      name="qkvW_sb", bufs=2)
                nc.sync.dma_start(
                    out=w["qkv"][:, :, :],
                    in_=qkvW[l].rearrange("(k p) n -> p k n", p=P))
                w["proj"] = enc_w.tile([P, DT, D], BF16, tag="projW",
                                       name="projW_sb", bufs=1)
                nc.sync.dma_start(
                    out=w["proj"][:, :, :],
                    in_=projW[l].rearrange("(k p) n -> p k n", p=P))
                w["f1"] = enc_w.tile([P, DT, FF], BF16, tag="f1W",
                                     name="f1W_sb", bufs=1)
                nc.sync.dma_start(
                    out=w["f1"][:, :, :],
                    in_=f1W[l].rearrange("(k p) n -> p k n", p=P))
                w["f2"] = enc_w.tile([P, FFT, D], BF16, tag="f2W",
                                     name="f2W_sb", bufs=1)
                nc.sync.dma_start(
                    out=w["f2"][:, :, :],
                    in_=f2W[l].rearrange("(k p) n -> p k n", p=P))
                w["ebc"] = enc_pk.tile([P, 2, D], BF16, tag="ebc",
                                       name="ebc_sb", bufs=1)
                nc.sync.dma_start(
                    out=w["ebc"][:, :, :],
                    in_=_bcast(ebc[l].rearrange("(a b) -> a b", b=D)))
                w["epp"] = enc_pk.tile([P, 36], FP32, tag="epp",
                                       name="epp_sb", bufs=2)
                nc.sync.dma_start(out=w["epp"][:, :], in_=epp[l])
                # coef-scaled identities for the edge-bias matmul
                w["cI"] = enc_pk.tile([P, NH, P], BF16, tag="cI",
                                      name="cI_sb", bufs=2)
                for h in range(NH):
                    nc.vector.tensor_scalar(
                        w["cI"][:, h, :], self.ident32[:, :],
                        w["epp"][:, 28 + h:29 + h], None, op0=ALU.mult)
                return w

            cur = load_enc(0)
            load_hr()
            nc.sync.dma_start(
                out=gbc_en[:, :, :],
                in_=_bcast(gbc[3 * D:5 * D].rearrange("(a b) -> a b", b=D)))
            nc.sync.dma_start(
                out=up1W_sb[:, :, :],
                in_=up1W[:, :].rearrange("(k p) n -> p k n", p=P))
            nc.sync.dma_start(
                out=up2W_sb[:, :, :],
                in_=up2W[:, :].rearrange("(k p) n -> p k n", p=P))

            with pool_group(tc, [("enc_a1", 1, "SBUF"),
                                 ("enc_a2", 2, "SBUF")]) as (act1, act2):
                xa = [act2.tile([P, TE, D], BF16, tag="ln_out",
                                name=f"xa_{b}", bufs=3) for b in range(BE)]

                # ---------------- phase 0: input projection ----------------
                with pool_group(tc, [("ip_sb", 1, "SBUF"),
                                     ("ip_ps", 2, "PSUM"),
                                     ("ip_w", 1, "SBUF")]) \
                        as (ip_sb, ip_ps, ip_w):
                    gbc_ip = ip_w.tile([P, 3, D], BF16)
                    nc.scalar.dma_start(
                        out=gbc_ip[:, :, :],
                        in_=_bcast(gbc[0:3 * D].rearrange("(a b) -> a b",
                                                          b=D)))
                    ipW_sb = ip_w.tile([P, TE, D], BF16)
                    nc.scalar.dma_start(
                        out=ipW_sb[:, :, :],
                        in_=ipW[:, :].rearrange("(k p) n -> p k n", p=P))
                    x_sbs = []
                    for b in range(BE):
                        x_sb = ip_sb.tile([P, TE, NLR], BF16, tag=f"x{b}",
                                          name=f"x{b}")
                        nc.scalar.dma_start(
                            out=x_sb[:, :, :],
                            in_=x_in[b].rearrange("(t p) m -> p t m", p=P))
                        x_sbs.append(x_sb)
                    zs = []
                    for b in range(BE):
                        xt = ip_sb.tile([P, TE, NLR], BF16, tag="xt",
                                        name="xt")
                        self.transpose_group(
                            ip_ps,
                            lambda t, f, b=b:
                                x_sbs[b][:, t, f * P:(f + 1) * P],
                            TE, TE, xt)
                        z = ip_sb.tile([P, TE, D], BF16, tag=f"z{b}",
                                       name=f"z{b}")
                        for m in range(TE):
                            ps = ip_ps.tile([P, D], FP32, tag="mm",
                                            name="ps")
                            self.mm(ps[:, :],
                                    lambda k: xt[:, k, m * P:(m + 1) * P],
                                    lambda k: ipW_sb[:, k, :], TE)
                            nc.vector.tensor_tensor(z[:, m, :], ps[:, :],
                                                    gbc_ip[:, 0, :],
                                                    op=ALU.add)
                        zs.append(z)
                    for b in range(BE):
                        lns = ip_sb.tile([P, TE, D], BF16, tag=f"lnout{b}",
                                         name=f"lnout{b}")
                        mvs_z = self.make_stats(
                            (lambda t, z=zs[b]: z[:, t, :]), TE)
                        self.emit_ln(mvs_z,
                                     (lambda t, z=zs[b]: z[:, t, :]),
                                     lns, TE,
                                     gbc_ip[:, 1, :], gbc_ip[:, 2, :])
                        mvs_h = self.small.tile([P, TE, 2], FP32,
                                                tag=f"mvh{b}",
                                                name=f"mvh{b}", bufs=4)
                        for t in range(TE):
                            nc.scalar.activation(h_enc[b][:, t, :],
                                                 lns[:, t, :], AF.Gelu)
                            self.emit_stats(mvs_h, t, h_enc[b][:, t, :])
                        self.emit_ln(mvs_h,
                                     (lambda t, b=b: h_enc[b][:, t, :]),
                                     xa[b], TE)

                # ---------------- encoder layers ----------------
                ly_ctx = ExitStack()
                ly_pools = ly_ctx.enter_context(pool_group(
                    tc, [("ly_mm", 2, "PSUM"), ("ly_s", 2, "PSUM"),
                         ("ly_o", 2, "PSUM"), ("ly_tr", 2, "PSUM")]))
                for l in range(L):
                    w = cur
                    if l + 1 < L:
                        cur = load_enc(l + 1)
                    xf = [act2.tile([P, TE, D], BF16, tag="ln_out",
                                    name=f"xf{l}_{b}", bufs=3)
                          for b in range(BE)]
                    self.attn_phase(
                        act1, act2, TE, h_enc, xa,
                        w["qkv"], w["proj"],
                        qkvb_cols=w["epp"][:, 0:12],
                        projb=w["ebc"][:, 0, :],
                        a_list=a_t, cI=w["cI"],
                        emit_next=lambda b, mvs, xf=xf: self.emit_ln(
                            mvs, (lambda t, b=b: h_enc[b][:, t, :]),
                            xf[b], TE),
                        pools=ly_pools)
                    if l + 1 < L:
                        xa = [act2.tile([P, TE, D], BF16, tag="ln_out",
                                        name=f"xa{l+1}_{b}", bufs=3)
                              for b in range(BE)]

                        def mk_next(xa=xa):
                            return lambda b, mvs: self.emit_ln(
                                mvs, (lambda t, b=b: h_enc[b][:, t, :]),
                                xa[b], TE)
                        nxt = mk_next()
                    else:
                        def nxt(b, mvs):
                            self.emit_ln(
                                mvs, (lambda t, b=b: h_enc[b][:, t, :]),
                                hfs[b], TE,
                                gbc_en[:, 0, :], gbc_en[:, 1, :])
                    self.ffn_phase(
                        act1, act2, TE, h_enc, xf, w["f1"], w["f2"],
                        f1b_cols=w["epp"][:, 12:28], f2b=w["ebc"][:, 1, :],
                        emit_next=nxt, pools=ly_pools)

            enc_w_ctx.close()

            # ---------------- upsample (hfs ready from enc tail) ----------
            if True:
                up_ps = ly_pools[0]
                for b in range(BE):
                    mvs_u = self.small.tile([P, TH, 2], FP32, tag=f"mvr{b}",
                                            name=f"mvr{b}", bufs=4)
                    g1 = up_sb.tile([P, TH, D], BF16, tag="g1", name="g1")
                    for mh in range(TH):
                        ps = up_ps.tile([P, D], FP32, tag="mm", name="ps")
                        self.mm(ps[:, :],
                                lambda k: up1W_sb[:, k, mh * P:(mh + 1) * P],
                                lambda k: hfs[b][:, k, :], TE)
                        nc.scalar.activation(g1[:, mh, :], ps[:, :], AF.Gelu,
                                             bias=gpp_sb[:, mh:mh + 1])
                    for mh in range(TH):
                        ps = up_ps.tile([P, D], FP32, tag="mm", name="ps")
                        self.mm(ps[:, :],
                                lambda k: up2W_sb[:, k, mh * P:(mh + 1) * P],
                                lambda k: g1[:, k, :], TH)
                        nc.vector.tensor_scalar(
                            h_hr[b][:, mh, :], ps[:, :],
                            gpp_sb[:, 4 + mh:5 + mh], None, op0=ALU.add)
                        self.emit_stats(mvs_u, mh, h_hr[b][:, mh, :])
                    self.emit_ln(mvs_u,
                                 (lambda t, b=b: h_hr[b][:, t, :]),
                                 x_hr0[b], TH)
            self.ly_ctx = ly_ctx
            self.ly_pools = ly_pools
            up_ctx.close()

        # ---------------- HR refinement block ----------------
        with pool_group(tc, [("hr_w2", 1, "SBUF"), ("hr_a1", 1, "SBUF"),
                             ("hr_a2", 2, "SBUF")]) as (hr_w2, act1, act2):
            rf1W_sb = hr_w2.tile([P, DT, FF], BF16, tag="f1W")
            nc.sync.dma_start(
                out=rf1W_sb[:, :, :],
                in_=rf1W[:, :].rearrange("(k p) n -> p k n", p=P))
            rf2W_sb = hr_w2.tile([P, FFT, D], BF16, tag="f2W")
            nc.sync.dma_start(
                out=rf2W_sb[:, :, :],
                in_=rf2W[:, :].rearrange("(k p) n -> p k n", p=P))
            xr = [act2.tile([P, TH, D], BF16, tag="ln_out", name=f"xr_{b}",
                            bufs=3) for b in range(BE)]
            self.attn_phase(
                act1, act2, TH, h_hr, x_hr0, rqkvW_sb, rprojW_sb,
                qkvb_cols=gpp_sb[:, 8:20],
                projb=gbc_hr[:, 0, :],
                emit_next=lambda b, mvs: self.emit_ln(
                    mvs, (lambda t, b=b: h_hr[b][:, t, :]), xr[b], TH),
                pools=self.ly_pools)
            self.ly_ctx.close()
            hf_ctx = ExitStack()
            hf_pools = hf_ctx.enter_context(pool_group(
                tc, [("hf_ps", 2, "PSUM"), ("hf_acc", 1, "PSUM"),
                     ("hf_tr", 2, "PSUM")]))
            self.hf_ctx = hf_ctx
            self.hf_pools = hf_pools
            self.ffn_phase(
                act1, act2, TH, h_hr, xr, rf1W_sb, rf2W_sb,
                f1b_cols=gpp_sb[:, 20:36], f2b=gbc_hr[:, 1, :],
                emit_next=lambda b, mvs: self.emit_ln(
                    mvs, (lambda t, b=b: h_hr[b][:, t, :]), hf2s[b], TH,
                    gbc_dec[:, 0, :], gbc_dec[:, 1, :]),
                pools=("ffn_own", hf_pools))

        # ---------------- decoder (hf2s ready from HR tail) ----------------
        with pool_group(tc, [("dec_sb", 1, "SBUF"),
                             ("dec_sb2", 2, "SBUF")]) as \
                (dec_sb, dec_sb2):
            fps, facc, trpool = self.hf_pools
            dec_ps = fps
            decW_sb = dec_sb.tile([P, KDEC, DT, D], BF16, tag="decW")
            nc.sync.dma_start(
                out=decW_sb[:, :, :, :],
                in_=decW[:, :, :].rearrange("kd (k p) m -> p kd k m", p=P))
            for b in range(BE):
                hft = dec_sb.tile([P, DT, NHR], BF16, tag="hft", name="hft")
                self.transpose_group(
                    trpool,
                    lambda t, f: hf2s[b][:, t, f * P:(f + 1) * P],
                    TH, DT, hft)
                m1t = dec_sb.tile([P, KDEC, DT, NHR], BF16, tag="m1t",
                                  name="m1t")
                for kd in range(KDEC):
                    for mi in range(DT):
                        ps = fps.tile([P, NHR], FP32, tag="s", name="ps")
                        self.mm(
                            ps[:, :],
                            lambda k, kd=kd, mi=mi:
                                decW_sb[:, kd, k, mi * P:(mi + 1) * P],
                            lambda k: hft[:, k, :], DT)
                        if (kd * DT + mi) % 2 == 0:
                            nc.vector.tensor_copy(m1t[:, kd, mi, :], ps[:, :])
                        else:
                            nc.scalar.copy(m1t[:, kd, mi, :], ps[:, :])
                out_sb = dec_sb2.tile([P, TH, NHR], FP32, tag="out",
                                      name="out_sb")
                for md in range(TH):
                    ncols = NHR - md * P
                    ps = facc.tile([P, NHR], FP32, tag=f"facc{md % 2}",
                                   name="ps_ak")
                    cnt = 0
                    for kd in range(KDEC):
                        for k in range(DT):
                            nc.tensor.matmul(
                                ps[:, 0:ncols],
                                m1t[:, kd, k, md * P:(md + 1) * P],
                                hft[:, k, md * P:],
                                start=(cnt == 0),
                                stop=(cnt == KDEC * DT - 1))
                            cnt += 1
                    # softplus(x/K + b) = ln(1 + exp(x/K + b))
                    sp_e = self.mid.tile([P, NHR], FP32, tag="sp_e",
                                         name="sp_e")
                    nc.scalar.activation(sp_e[:, 0:ncols], ps[:, 0:ncols],
                                         AF.Exp,
                                         bias=gpp_sb[:, 36:37],
                                         scale=1.0 / KDEC)
                    nc.scalar.activation(out_sb[:, md, 0:ncols],
                                         sp_e[:, 0:ncols],
                                         AF.Ln, bias=1.0)
                    nc.sync.dma_start(
                        out=out_d[b].rearrange("(t p) m -> p t m", p=P)
                            [:, md, md * P:],
                        in_=out_sb[:, md, 0:ncols])
            self.hf_ctx.close()

    # ---- attention phase (both batch elems) -------------------------------
    def attn_phase(self, act1, act2, T, h_list, x1s, qkvW_sb, projW_sb,
                   qkvb_cols, projb, a_list=None, cI=None, emit_next=None,
                   pools=None):
        nc = self.nc
        tc = self.tc
        N = T * P
        if T == TE:
            ps_specs = [("at_ps", 2, "PSUM"), ("at_s", 2, "PSUM"),
                        ("at_o", 2, "PSUM"), ("at_tr", 2, "PSUM")]
        else:
            ps_specs = [("at_ps", 2, "PSUM"), ("at_s", 3, "PSUM"),
                        ("at_o", 2, "PSUM"), ("at_tr", 1, "PSUM")]
        tr_bufs = 2 if (T == TE or pools is not None) else 1
        ctx = ExitStack()
        if pools is None:
            aps, spool, opool, trpool = ctx.enter_context(
                pool_group(tc, ps_specs))
        else:
            aps, spool, opool, trpool = pools
        with ctx:
            x1t = []
            vexts = []
            qks = []
            for b in range(BE):
                xt = act2.tile([P, DT, N], BF16, tag="ln_t", name="x1t")
                self.transpose_group(
                    trpool, lambda t, f: x1s[b][:, t, f * P:(f + 1) * P],
                    T, DT, xt, ps_bufs=tr_bufs)
                x1t.append(xt)
                # V token-major: psum [tokens, 8*64] -> vext with ones cols
                vext = act2.tile([P, T, NH, HD + 4], BF16, tag="vext",
                                 name=f"vext{b}", bufs=2)
                nc.vector.memset(vext[:, :, :, 0:4], 1.0)
                for t in range(T):
                    ps = aps.tile([P, D], FP32, tag="mm", name="ps_v")
                    self.mm(ps[:, :],
                            lambda k, t=t: x1t[b][:, k, t * P:(t + 1) * P],
                            lambda k: qkvW_sb[:, k, 2 * D:3 * D], DT)
                    ps_h = ps[:, :].rearrange("p (h c) -> p h c", c=HD)
                    if t % 2 == 0:
                        nc.scalar.copy(vext[:, t, :, 4:], ps_h)
                    else:
                        nc.vector.tensor_copy(vext[:, t, :, 4:], ps_h)
                vexts.append(vext)
                # Q/K feature-major per head-pair
                qk = act2.tile([P, DT, 2, N], BF16, tag="qk", name=f"qk{b}",
                               bufs=2)
                for hp in range(NH // 2):
                    for j, mi in enumerate((hp, 4 + hp)):
                        ps = aps.tile([P, N], FP32, tag="mm", name="ps_qk")
                        self.mm(
                            ps[:, :],
                            lambda k, mi=mi:
                                qkvW_sb[:, k, mi * P:(mi + 1) * P],
                            lambda k: x1t[b][:, k, :], DT)
                        if j == 0:  # q: (x + bias) * hd^-0.5
                            nc.vector.tensor_scalar(
                                qk[:, hp, j, :], ps[:, :],
                                qkvb_cols[:, mi:mi + 1], HD ** -0.5,
                                op0=ALU.add, op1=ALU.mult)
                        else:
                            nc.vector.tensor_scalar(
                                qk[:, hp, j, :], ps[:, :],
                                qkvb_cols[:, mi:mi + 1], None, op0=ALU.add)
                qks.append(qk)
            # scores + O, pipelined per head-pair: emit scores(hp), then
            # O(hp-1) — the ScalarE exp of pair hp overlaps PE O of hp-1.
            o_sbs = []
            for b in range(BE):
                o_sb = act1.tile([P, T, D], BF16, tag="o_sb", name=f"o_sb{b}",
                                 bufs=2)
                o_sbs.append(o_sb)

            def emit_scores(b, hp):
                pt = act1.tile([P, 2, T, N], BF16, tag="pT",
                               name=f"pt{b}_{hp}", bufs=3)
                for hh in range(2):
                    h_idx = 2 * hp + hh
                    base = hh * HD
                    qa = qks[b][base:base + HD, hp, 0, :]
                    ka = qks[b][base:base + HD, hp, 1, :]
                    if T == TE:
                        ps_s = spool.tile([P, T, N], FP32, tag="s",
                                          name="ps_s")
                        for kk in range(T):
                            if a_list is not None:
                                nc.tensor.matmul(
                                    ps_s[:, kk, :],
                                    cI[:, h_idx, :], a_list[b][:, kk, :],
                                    start=(kk == 0), stop=False)
                                nc.tensor.matmul(
                                    ps_s[:, kk, :],
                                    ka[:, kk * P:(kk + 1) * P], qa,
                                    start=False, stop=(kk == T - 1))
                            else:
                                nc.tensor.matmul(
                                    ps_s[:, kk, :],
                                    ka[:, kk * P:(kk + 1) * P], qa,
                                    start=(kk == 0), stop=(kk == T - 1))
                        nc.scalar.activation(pt[:, hh, :, :],
                                             ps_s[:, :, :], AF.Exp)
                    else:
                        for kk in range(T):
                            ps_s = spool.tile([P, N], FP32, tag="s",
                                              name="ps_s", bufs=2)
                            nc.tensor.matmul(
                                ps_s[:, :],
                                ka[:, kk * P:(kk + 1) * P], qa,
                                start=True, stop=True)
                            nc.scalar.activation(
                                pt[:, hh, kk, :],
                                ps_s[:, :], AF.Exp)
                return pt

            def emit_o(b, hp, pt):
                for hh in range(2):
                    h_idx = 2 * hp + hh
                    ps_o = opool.tile([P, T, HD + 4], FP32, tag="o",
                                      name="ps_o")
                    for m in range(T):
                        for kk in range(T):
                            nc.tensor.matmul(
                                ps_o[:, m, :],
                                pt[:, hh, kk, m * P:(m + 1) * P],
                                vexts[b][:, kk, h_idx, :],
                                start=(m == 0 and kk == 0),
                                stop=(m == T - 1 and kk == T - 1))
                    rinv = self.small.tile([P, T, 1], FP32, tag="rinv",
                                           name="rinv", bufs=4)
                    nc.vector.reciprocal(rinv[:, :, :], ps_o[:, :, 0:1])
                    for m in range(T):
                        if (h_idx + m) % 2 == 0:
                            nc.scalar.mul(
                                o_sbs[b][:, m, h_idx * HD:(h_idx + 1) * HD],
                                ps_o[:, m, 4:], rinv[:, m, 0:1])
                        else:
                            nc.vector.tensor_scalar(
                                o_sbs[b][:, m, h_idx * HD:(h_idx + 1) * HD],
                                ps_o[:, m, 4:], rinv[:, m, 0:1], None,
                                op0=ALU.mult)

            prev = None
            for b in range(BE):
                for hp in range(NH // 2):
                    pt = emit_scores(b, hp)
                    if prev is not None:
                        emit_o(prev[0], prev[1], prev[2])
                    prev = (b, hp, pt)
            emit_o(prev[0], prev[1], prev[2])
            # o -> feature-major oT, then proj + residual (+ eager LN stats)
            mvs_out = [self.small.tile([P, T, 2], FP32, tag=f"mva{b}",
                                       name=f"mva{b}", bufs=4)
                       for b in range(BE)]
            for b in range(BE):
                ot = act1.tile([P, DT, N], BF16, tag="oT", name="ot")
                self.transpose_group(
                    trpool, lambda t, f: o_sbs[b][:, t, f * P:(f + 1) * P],
                    T, DT, ot, ps_bufs=tr_bufs)
                for m in range(T):
                    ps = aps.tile([P, D], FP32, tag="mm", name="ps_proj")
                    self.mm(ps[:, :],
                            lambda k: ot[:, k, m * P:(m + 1) * P],
                            lambda k: projW_sb[:, k, :], DT)
                    nc.vector.tensor_tensor(h_list[b][:, m, :],
                                            h_list[b][:, m, :], ps[:, :],
                                            op=ALU.add)
                    nc.vector.tensor_tensor(h_list[b][:, m, :],
                                            h_list[b][:, m, :], projb,
                                            op=ALU.add)
                    self.emit_stats(mvs_out[b], m, h_list[b][:, m, :])
                if emit_next is not None:
                    emit_next(b, mvs_out[b])

    # ---- FFN phase (both batch elems) -------------------------------------
    def ffn_phase(self, act1, act2, T, h_list, x2s, f1W_sb, f2W_sb,
                  f1b_cols, f2b, emit_next=None, pools=None):
        nc = self.nc
        tc = self.tc
        N = T * P
        ctx = ExitStack()
        if pools is None:
            fps, facc, trpool = ctx.enter_context(pool_group(
                tc, [("ff_ps", 2, "PSUM"), ("ff_acc", 1, "PSUM"),
                     ("ff_tr", 2, "PSUM")]))
            facc_tag = lambda m: (facc, f"facc{m}")
        elif isinstance(pools, tuple) and pools[0] == "ffn_own":
            fps, facc, trpool = pools[1]
            facc_tag = lambda m: (facc, f"facc{m}")
        else:
            aps, spool, opool, trpool = pools
            fps = spool
            facc_tag = lambda m: (opool, "o")
        with ctx:
            mvs_out = [self.small.tile([P, T, 2], FP32, tag=f"mvf{b}",
                                       name=f"mvf{b}", bufs=4)
                       for b in range(BE)]
            x2t = {}
            for b in range(BE):
                xt = act2.tile([P, DT, N], BF16, tag="ln_t", name="x2t")
                self.transpose_group(
                    trpool, lambda t, f: x2s[b][:, t, f * P:(f + 1) * P],
                    T, DT, xt)
                x2t[b] = xt
                ps_f2 = [facc_tag(m)[0].tile([P, D], FP32,
                                             tag=facc_tag(m)[1],
                                             name=f"facc{m}")
                         for m in range(T)]
                half = FFT // 4
                for wave in range(4):
                    gt = act1.tile([P, half, N], BF16, tag="gT", name="gt")
                    for j in range(half):
                        mf = wave * half + j
                        ps = fps.tile([P, N], FP32, tag="s", name="ps_f1")
                        self.mm(
                            ps[:, :],
                            lambda k, mf=mf:
                                f1W_sb[:, k, mf * P:(mf + 1) * P],
                            lambda k: x2t[b][:, k, :], DT)
                        nc.scalar.activation(gt[:, j, :], ps[:, :], AF.Gelu,
                                             bias=f1b_cols[:, mf:mf + 1])
                    for m in range(T):
                        for j in range(half):
                            mf = wave * half + j
                            nc.tensor.matmul(
                                ps_f2[m][:, :], gt[:, j, m * P:(m + 1) * P],
                                f2W_sb[:, mf, :],
                                start=(mf == 0), stop=(mf == FFT - 1))
                for m in range(T):
                    nc.vector.tensor_tensor(h_list[b][:, m, :],
                                            h_list[b][:, m, :],
                                            ps_f2[m][:, :], op=ALU.add)
                    nc.vector.tensor_tensor(h_list[b][:, m, :],
                                            h_list[b][:, m, :], f2b,
                                            op=ALU.add)
                    self.emit_stats(mvs_out[b], m, h_list[b][:, m, :])
                if emit_next is not None:
                    emit_next(b, mvs_out[b])


# --------------------------------------------------------------------------
# host-side driver
# --------------------------------------------------------------------------
_CACHE = {}
_TRIU = np.triu_indices(NHR, k=1)


def _np(x):
    return np.ascontiguousarray(np.asarray(x, dtype=np.float32))


def _bf(x):
    import ml_dtypes
    return np.ascontiguousarray(np.asarray(x).astype(ml_dtypes.bfloat16))


def kernel(**inputs):
    res = run_on_device(inputs)
    full = np.concatenate([res.results[c]["OUT"] for c in range(NCORES)],
                          axis=0)  # (16, 512, 512)
    return np.ascontiguousarray(full[:, _TRIU[0], _TRIU[1]]).astype(np.float32)


def _fold_ln(g, b, w, bias):
    """(xn*g + b) @ w + bias  ==  xn @ (diag(g) w) + (bias + b @ w)."""
    w64 = w.astype(np.float64)
    w2 = (g.astype(np.float64)[:, None] * w64).astype(np.float32)
    b2 = (bias.astype(np.float64) + b.astype(np.float64) @ w64).astype(
        np.float32)
    return w2, b2


def run_on_device(inputs, **run_kwargs):
    if "nc" not in _CACHE:
        _CACHE["nc"] = build_nc()
    nc = _CACHE["nc"]

    inp = {k: _np(v) for k, v in inputs.items()}

    qkvW_f = np.empty_like(inp["e_qkvW"])
    qkvb_f = np.empty_like(inp["e_qkvb"])
    f1W_f = np.empty_like(inp["e_f1W"])
    f1b_f = np.empty_like(inp["e_f1b"])
    for l in range(L):
        qkvW_f[l], qkvb_f[l] = _fold_ln(inp["e_n1g"][l], inp["e_n1b"][l],
                                        inp["e_qkvW"][l], inp["e_qkvb"][l])
        f1W_f[l], f1b_f[l] = _fold_ln(inp["e_n2g"][l], inp["e_n2b"][l],
                                      inp["e_f1W"][l], inp["e_f1b"][l])
    rqkvW_f, rqkvb_f = _fold_ln(inp["r_n1g"], inp["r_n1b"],
                                inp["r_qkvW"], inp["r_qkvb"])
    rf1W_f, rf1b_f = _fold_ln(inp["r_n2g"], inp["r_n2b"],
                              inp["r_f1W"], inp["r_f1b"])

    # fold the v bias into the proj bias: softmax rows sum to 1, so
    # o_full = o_norm + bv and (o+bv)@W + b = o@W + (b + bv@W).
    eprojb_f = np.empty_like(inp["e_projb"])
    for l in range(L):
        eprojb_f[l] = (inp["e_projb"][l].astype(np.float64)
                       + qkvb_f[l][2 * D:].astype(np.float64)
                       @ inp["e_projW"][l].astype(np.float64)).astype(
            np.float32)
    rprojb_f = (inp["r_projb"].astype(np.float64)
                + rqkvb_f[2 * D:].astype(np.float64)
                @ inp["r_projW"].astype(np.float64)).astype(np.float32)

    ebc = np.stack([
        np.concatenate([eprojb_f[l], inp["e_f2b"][l]])
        for l in range(L)
    ])
    epp = np.stack([
        np.concatenate([
            qkvb_f[l].reshape(12, P).T,
            f1b_f[l].reshape(FFT, P).T,
            np.broadcast_to(inp["e_ebs"][l] * inp["e_ebW"][l], (P, NH)),
        ], axis=1)
        for l in range(L)
    ])
    gbc = np.concatenate([
        inp["ip_b"], inp["ip_g"], inp["ip_bt"], inp["encn_g"], inp["encn_b"],
        rprojb_f, inp["r_f2b"], inp["hrn_g"], inp["hrn_b"],
    ])
    gpp = np.concatenate([
        inp["up1b"].reshape(TH, P).T,
        inp["up2b"].reshape(TH, P).T,
        rqkvb_f.reshape(12, P).T,
        rf1b_f.reshape(FFT, P).T,
        np.broadcast_to(inp["dec_b"][0], (P, 1)),
    ], axis=1)
    dec_sym = 0.5 * (inp["dec_W"] + inp["dec_W"].transpose(0, 2, 1))
    # the transposed-score path uses A^T == A; guarantee symmetry
    a_sym = 0.5 * (inp["A_lr"] + inp["A_lr"].transpose(0, 2, 1))

    shared = {
        "ipW": _bf(inp["ip_W"]), "qkvW": _bf(qkvW_f),
        "projW": _bf(inp["e_projW"]),
        "f1W": _bf(f1W_f), "f2W": _bf(inp["e_f2W"]), "up1W": _bf(inp["up1W"]),
        "up2W": _bf(inp["up2W"]), "rqkvW": _bf(rqkvW_f),
        "rprojW": _bf(inp["r_projW"]),
        "rf1W": _bf(rf1W_f), "rf2W": _bf(inp["r_f2W"]),
        "decW": _bf(dec_sym),
        "ebc": _bf(ebc), "epp": np.ascontiguousarray(epp),
        "gbc": _bf(gbc), "gpp": np.ascontiguousarray(gpp),
    }
    in_maps = []
    for c in range(NCORES):
        m = dict(shared)
        m["X"] = _bf(inp["X_lr"][c * BE:(c + 1) * BE])
        m["AB"] = _bf(a_sym[c * BE:(c + 1) * BE])
        in_maps.append(m)

    return run_bass_kernel_spmd(nc, in_maps, list(range(NCORES)), **run_kwargs)


if __name__ == "__main__":
    import time
    t0 = time.time()
    nc = build_nc()
    print(f"build+finalize: {time.time() - t0:.1f}s, insts={len(nc.inst_map)}")
